# revision 31
# baseline (speedup 1.0000x reference)
"""MaskAwareTransformer Trainium2 kernel.

Sharding: 8 cores = 4 batch items x 2 sequence halves.
Per core: feature-major activations (h^T = [D, tok]); host-pretransposed
weights; attention computed as S^T = [keys, queries] so the key-mask folds
into the exp bias (per-partition) and key-sums become ones-column matmuls;
per-layer 2-rank AllGather exchanges updated sequence halves.
"""

import numpy as np
import ml_dtypes
from contextlib import ExitStack

import concourse.bass as bass
import concourse.bacc as bacc
import concourse.mybir as mybir
import concourse.tile as tile
from concourse.bass_utils import run_bass_kernel_spmd

F32 = mybir.dt.float32
BF = mybir.dt.bfloat16
AF = mybir.ActivationFunctionType
ALU = mybir.AluOpType

# Problem constants
B = 4
PATCH = 8
NH = 8
L = 8
D = 512
DF = 2048
NTOK = 1024        # patches per image
NOWN = 512         # tokens per core
CPP = 192          # 3*8*8 patch features
MPP = 64           # 1*8*8 mask patch features
DK = 64
DT = D // 128      # 4 feature tiles
KT8 = NTOK // 128  # 8 key tiles
EPS = 1e-5
SCALE = 1.0 / 8.0  # 1/sqrt(dk)
MASK_NEG = -30.0
PAIRS = [[0, 1], [2, 3], [4, 5], [6, 7]]


def _build(nc: bass.Bass):
    # ---------------- DRAM I/O ----------------
    def inp(name, shape, dt=F32):
        return nc.dram_tensor(name, list(shape), dt, kind="ExternalInput").ap()

    xf_d = inp("xpT_f", (CPP, NTOK), BF)
    xo_d = inp("xpT_o", (CPP, NOWN), BF)
    mf_d = inp("mpT_f", (MPP, NTOK), BF)
    mo_d = inp("mpT_o", (MPP, NOWN), BF)
    posf_d = inp("posT_f", (D, NTOK))
    poso_d = inp("posT_o", (D, NOWN))
    pmb_d = inp("pmb", (NTOK,))
    pw_d = inp("patch_wT", (CPP, D), BF)
    mw_d = inp("mask_wT", (MPP, D), BF)
    be_d = inp("bias_embed", (D,))
    wq_d = inp("WqT", (L, D, D), BF)
    wk_d = inp("WkT", (L, D, D), BF)
    wv_d = inp("WvT", (L, D, D), BF)
    wo_d = inp("WoT", (L, D, D), BF)
    w1_d = inp("W1T", (L, D, DF), BF)
    w2_d = inp("W2T", (L, DF, D), BF)
    bq_d = inp("bq", (L, D))
    bk_d = inp("bk", (L, D))
    bv_d = inp("bv", (L, D), BF)
    bo_d = inp("bo", (L, D))
    b1_d = inp("b1", (L, DF))
    b2_d = inp("b2", (L, D))
    g1_d = inp("g1", (L, D), BF)
    be1_d = inp("be1", (L, D), BF)
    g2_d = inp("g2", (L, D), BF)
    be2_d = inp("be2", (L, D), BF)
    ow_d = inp("out_wT", (D, CPP), BF)
    ob_d = inp("out_b", (CPP,))
    out_d = nc.dram_tensor("oT", [CPP, NOWN], F32, kind="ExternalOutput").ap()

    ctx = ExitStack()
    tc = tile.TileContext(nc)
    tc.__enter__()

    # ---------------- pools ----------------
    def sbp(name, bufs):
        return ctx.enter_context(tc.tile_pool(name=name, bufs=bufs))

    def psp(name, bufs):
        return ctx.enter_context(tc.tile_pool(name=name, bufs=bufs, space="PSUM"))

    pool_const = sbp("const", 1)
    pool_wqkv = sbp("wqkv", 8)    # [128,512] shared weight/staging slots
    pool_wo = sbp("wo", 4)
    pool_w2 = sbp("w2", 4)
    pool_hF = sbp("hF", 4)        # [128,1024]
    pool_hO = sbp("hO", 6)
    pool_QT = sbp("QT", 4)
    pool_KT = sbp("KT", 4)        # [128,1024]
    pool_vst = sbp("vst", 1)
    pool_E = sbp("E", 7)
    pool_ctx = sbp("ctx", 4)
    pool_res = sbp("res", 4)
    pool_sq = sbp("sq", 4)
    pool_aT = sbp("aT", 4)
    pool_u = sbp("u", 8)
    pool_rows = sbp("rows", 2)
    pool_bias = sbp("bias", 1)
    pool_out = sbp("outp", 2)

    ps_gen = psp("ps_gen", 4)   # generic rotating PSUM slots (tag "ps")
    ps_f = psp("ps_f", 4)       # FFN second-gemm accumulators (held)
    ps_m = ps_s = ps_c = ps_r = ps_ab = ps_u = ps_gen
    dram = ctx.enter_context(tc.tile_pool(name="dram", bufs=1, space="DRAM"))

    def wtile(nm):
        return pool_wqkv.tile([128, 512], BF, name=nm, tag="wqkv")

    # ---------------- constants ----------------
    ones_row = pool_const.tile([1, 512], BF, name="ones_row")
    nc.vector.memset(ones_row, 1.0)
    ones_col = pool_const.tile([128, 1], BF, name="ones_col")
    nc.vector.memset(ones_col, 1.0)
    eps_tile = pool_const.tile([1, 1], F32, name="eps_tile")
    nc.vector.memset(eps_tile, EPS)
    pmb_sb = pool_const.tile([128, KT8], F32, name="pmb_sb")
    nc.sync.dma_start(pmb_sb, pmb_d.rearrange("(t p) -> p t", p=128))

    # V token-major storage with per-head ones column (for Z)
    vst = []
    for tt in range(KT8):
        v_t = pool_vst.tile([128, NH * 65], BF, name=f"vst{tt}", tag=f"vst{tt}")
        for h in range(NH):
            nc.vector.memset(v_t[:, h * 65 + 64:h * 65 + 65], 1.0)
        vst.append(v_t)

    def bias_cols(name, dvec, ncols):
        """[n] DRAM vector -> [128, n/128] SBUF (per-partition bias columns)."""
        t = pool_bias.tile([128, ncols], F32, name=name, tag=name)
        nc.sync.dma_start(t, dvec.rearrange("(c p) -> p c", p=128))
        return t

    # ---------------- embed ----------------
    pw0 = wtile("pw0")
    nc.sync.dma_start(pw0, pw_d[0:128, :])
    pw1 = wtile("pw1")
    nc.sync.dma_start(pw1[0:64, :], pw_d[128:192, :])
    mw = wtile("mw")
    nc.sync.dma_start(mw[0:64, :], mw_d)
    be_sb = bias_cols("be_sb", be_d, DT)

    hF = [pool_hF.tile([128, NTOK], BF, name="hF", tag="hF") for _ in range(DT)]
    hO = []

    def embed_half(x_dram, m_dram, pos_dram, dst_list, dst_off):
        x0 = wtile("x0")
        nc.sync.dma_start(x0, x_dram[0:128, :])
        x1 = wtile("x1")
        nc.sync.dma_start(x1[0:64, :], x_dram[128:192, :])
        mm_t = wtile("mm_t")
        nc.sync.dma_start(mm_t[0:64, :], m_dram)
        for c in range(DT):
            cs = slice(c * 128, (c + 1) * 128)
            pse = ps_m.tile([128, 512], F32, name="pse", tag="ps")
            nc.tensor.matmul(pse, pw0[:, cs], x0, start=True, stop=False)
            nc.tensor.matmul(pse, pw1[0:64, cs], x1[0:64, :],
                             start=False, stop=False)
            nc.tensor.matmul(pse, mw[0:64, cs], mm_t[0:64, :],
                             start=False, stop=True)
            pos_t = pool_wqkv.tile([128, 512], F32, name="pos_t", tag="pose",
                                   bufs=2)
            nc.sync.dma_start(pos_t, pos_dram[cs, :])
            nc.vector.scalar_tensor_tensor(
                out=dst_list[c][:, dst_off:dst_off + 512], in0=pse,
                scalar=be_sb[:, c:c + 1], in1=pos_t,
                op0=ALU.add, op1=ALU.add)

    for t in range(2):
        tsl = slice(t * 512, (t + 1) * 512)
        embed_half(xf_d[:, tsl], mf_d[:, tsl], posf_d[:, tsl], hF, t * 512)
    hO = [pool_hO.tile([128, NOWN], F32, name="hO", tag="hO") for _ in range(DT)]
    embed_half(xo_d, mo_d, poso_d, hO, 0)
    hOb = []
    for c in range(DT):
        hOb_c = pool_hO.tile([128, NOWN], BF, name="hOb", tag="hOb")
        nc.vector.tensor_copy(hOb_c, hO[c])
        hOb.append(hOb_c)

    # ---------------- layernorm helper ----------------
    def layernorm(src_tiles, srcb_tiles, gb, out_pool, out_tag):
        sq_tiles = []
        for c in range(DT):
            sq_c = pool_sq.tile([128, 512], BF, name="sq", tag="sq")
            nc.scalar.activation(sq_c, src_tiles[c], AF.Square)
            sq_tiles.append(sq_c)
        psum_mu = ps_r.tile([1, 512], F32, name="psmu", tag="ps")
        psum_ms = ps_r.tile([1, 512], F32, name="psms", tag="ps")
        for c in range(DT):
            nc.tensor.matmul(psum_mu, ones_col, srcb_tiles[c],
                             start=(c == 0), stop=(c == DT - 1))
        for c in range(DT):
            nc.tensor.matmul(psum_ms, ones_col, sq_tiles[c],
                             start=(c == 0), stop=(c == DT - 1))
        # All row vectors at base partition 0 (engine ops require equal
        # SBUF base partitions; matmul lhsT/rhs must share base too).
        mu = pool_rows.tile([1, 512], F32, name="mu", tag="mu", bufs=2)
        ms = pool_rows.tile([1, 512], F32, name="ms", tag="ms", bufs=2)
        mu2 = pool_rows.tile([1, 512], F32, name="mu2", tag="mu2", bufs=2)
        var = pool_rows.tile([1, 512], F32, name="var", tag="var", bufs=2)
        std = pool_rows.tile([1, 512], F32, name="std", tag="std", bufs=1)
        rstd = pool_rows.tile([1, 512], BF, name="rstd", tag="rstd", bufs=1)
        nmr = pool_rows.tile([1, 512], BF, name="nmr", tag="nmr", bufs=1)
        nc.vector.tensor_scalar_mul(mu, psum_mu, 1.0 / D)
        nc.vector.tensor_scalar_mul(ms, psum_ms, 1.0 / D)
        nc.vector.tensor_mul(mu2, mu, mu)
        nc.vector.tensor_sub(var, ms, mu2)
        nc.scalar.activation(std, var, AF.Sqrt, bias=eps_tile)
        with nc.allow_low_precision(reason="rstd to bf16 for bcast matmul"):
            nc.vector.reciprocal(rstd, std)
        nc.vector.scalar_tensor_tensor(
            out=nmr, in0=mu, scalar=-1.0, in1=rstd, op0=ALU.mult, op1=ALU.mult)
        g_row, be_row = gb
        outs = []
        for c in range(DT):
            cs = slice(c * 128, (c + 1) * 128)
            psA = ps_ab.tile([128, 512], F32, name="psA", tag="ps")
            nc.tensor.matmul(psA, g_row[0:1, cs], rstd,
                             start=True, stop=True)
            psB = ps_ab.tile([128, 512], F32, name="psB", tag="ps")
            nc.tensor.matmul(psB, g_row[0:1, cs], nmr,
                             start=True, stop=False)
            nc.tensor.matmul(psB, be_row[0:1, cs], ones_row,
                             start=False, stop=True)
            o_c = out_pool.tile([128, 512], F32, name="lno", tag=out_tag)
            nc.vector.tensor_mul(o_c, src_tiles[c], psA)
            nc.vector.tensor_add(o_c, o_c, psB)
            outs.append(o_c)
        outsb = []
        for c in range(DT):
            ob_c = out_pool.tile([128, 512], BF, name="lnob", tag=out_tag + "b")
            nc.vector.tensor_copy(ob_c, outs[c])
            outsb.append(ob_c)
        return outs, outsb

    # ---------------- layers ----------------
    for l in range(L):
        bq_sb = bias_cols("bq_sb", bq_d[l], DT)
        bk_sb = bias_cols("bk_sb", bk_d[l], DT)
        bo_sb = bias_cols("bo_sb", bo_d[l], DT)
        b2_sb = bias_cols("b2_sb", b2_d[l], DT)
        b1_sb = bias_cols("b1_sb", b1_d[l], DF // 128)
        bv_row = pool_bias.tile([1, D], BF, name="bv_row", tag="bv_row")
        nc.sync.dma_start(bv_row, bv_d[l][None, :])

        def grow(nm, dvec):
            t = pool_bias.tile([1, D], BF, name=nm, tag=nm)
            nc.sync.dma_start(t, dvec[None, :])
            return t

        gb1 = (grow("g1r", g1_d[l]), grow("be1r", be1_d[l]))
        gb2 = (grow("g2r", g2_d[l]), grow("be2r", be2_d[l]))

        # -------- Q^T (own tokens) --------
        wq = []
        for kt in range(DT):
            w_t = wtile("wq")
            nc.sync.dma_start(w_t, wq_d[l, kt * 128:(kt + 1) * 128, :])
            wq.append(w_t)
        QT = []
        for c in range(DT):
            cs = slice(c * 128, (c + 1) * 128)
            psq = ps_m.tile([128, 512], F32, name="psq", tag="ps")
            for kt in range(DT):
                nc.tensor.matmul(psq, wq[kt][:, cs], hOb[kt],
                                 start=(kt == 0), stop=(kt == DT - 1))
            q_c = pool_QT.tile([128, 512], BF, name="qt", tag="QT")
            nc.scalar.activation(q_c, psq, AF.Identity, bias=bq_sb[:, c:c + 1])
            QT.append(q_c)

        # -------- K^T (all tokens) --------
        wk = []
        for kt in range(DT):
            w_t = wtile("wk")
            nc.sync.dma_start(w_t, wk_d[l, kt * 128:(kt + 1) * 128, :])
            wk.append(w_t)
        KTt = []
        for c in range(DT):
            cs = slice(c * 128, (c + 1) * 128)
            k_c = pool_KT.tile([128, NTOK], BF, name="ktile", tag="KT")
            for t in range(2):
                tsl = slice(t * 512, (t + 1) * 512)
                psk = ps_m.tile([128, 512], F32, name="psk", tag="ps")
                for kt in range(DT):
                    nc.tensor.matmul(psk, wk[kt][:, cs], hF[kt][:, tsl],
                                     start=(kt == 0), stop=(kt == DT - 1))
                nc.scalar.activation(k_c[:, tsl], psk, AF.Identity,
                                     bias=bk_sb[:, c:c + 1])
            KTt.append(k_c)

        # -------- V (token-major, all tokens) --------
        wv = []
        for kt in range(DT):
            w_t = wtile("wv")
            nc.sync.dma_start(w_t, wv_d[l, kt * 128:(kt + 1) * 128, :])
            wv.append(w_t)
        for tt in range(KT8):
            psv = ps_m.tile([128, 512], F32, name="psv", tag="ps")
            tok = slice(tt * 128, (tt + 1) * 128)
            for kt in range(DT):
                nc.tensor.matmul(psv, hF[kt][:, tok], wv[kt],
                                 start=(kt == 0), stop=False)
            nc.tensor.matmul(psv, ones_row[0:1, 0:128], bv_row,
                             start=False, stop=True)
            dst = vst[tt].rearrange("p (h w) -> p h w", w=65)[:, :, 0:64]
            nc.scalar.activation(dst, psv.rearrange("p (h w) -> p h w", w=64),
                                 AF.Copy)

        # -------- attention --------
        wo = []
        for kt in range(DT):
            w_t = pool_wo.tile([128, 512], BF, name="wot", tag="wo")
            nc.sync.dma_start(w_t, wo_d[l, kt * 128:(kt + 1) * 128, :])
            wo.append(w_t)
        ctxT = [pool_ctx.tile([128, 512], BF, name="ctxt", tag="ctx")
                for _ in range(DT)]
        zrows = []
        for h in range(NH):
            c = h // 2
            off = (h % 2) * DK
            E_tiles = []
            for kt in range(KT8):
                pss = ps_s.tile([128, 512], F32, name="pss", tag="ps")
                nc.tensor.matmul(
                    pss,
                    KTt[c][off:off + DK, kt * 128:(kt + 1) * 128],
                    QT[c][off:off + DK, :],
                    start=True, stop=True)
                e_kt = pool_E.tile([128, 512], BF, name="esc", tag="E")
                nc.scalar.activation(e_kt, pss, AF.Exp, scale=SCALE,
                                     bias=pmb_sb[:, kt:kt + 1])
                E_tiles.append(e_kt)
            psc = ps_c.tile([128, 512], F32, name="psc", tag="ps")
            for kt in range(KT8):
                nc.tensor.matmul(psc[0:65, :], vst[kt][:, h * 65:(h + 1) * 65],
                                 E_tiles[kt], start=(kt == 0),
                                 stop=(kt == KT8 - 1))
            nc.scalar.activation(ctxT[c][off:off + DK, :], psc[0:64, :], AF.Copy)
            z_h = pool_rows.tile([1, 512], F32, name="zrow", tag="zrow", bufs=8)
            nc.vector.tensor_copy(z_h, psc[64:65, :])
            zrows.append(z_h)

        rzrows = []
        for h in range(NH):
            rz_h = pool_rows.tile([1, 512], BF, name="rzrow", tag="rzrow", bufs=8)
            with nc.allow_low_precision(reason="1/Z to bf16 for bcast matmul"):
                nc.vector.reciprocal(rz_h, zrows[h])
            rzrows.append(rz_h)
        for c in range(DT):
            for sub in range(2):  # head 2c in ctxT[c][0:64], head 2c+1 in [64:128]
                h = 2 * c + sub
                psz = ps_ab.tile([128, 512], F32, name="psz", tag="ps")
                nc.tensor.matmul(psz[0:64, :], ones_row[0:1, 0:64],
                                 rzrows[h], start=True, stop=True)
                nc.vector.tensor_mul(ctxT[c][sub * 64:(sub + 1) * 64, :],
                                     ctxT[c][sub * 64:(sub + 1) * 64, :],
                                     psz[0:64, :])

        # -------- O-proj + residual --------
        a0 = []
        for c in range(DT):
            cs = slice(c * 128, (c + 1) * 128)
            pso = ps_m.tile([128, 512], F32, name="pso", tag="ps")
            for kt in range(DT):
                nc.tensor.matmul(pso, wo[kt][:, cs], ctxT[kt],
                                 start=(kt == 0), stop=(kt == DT - 1))
            a0_c = pool_res.tile([128, 512], F32, name="a0", tag="res")
            nc.vector.scalar_tensor_tensor(
                out=a0_c, in0=pso, scalar=bo_sb[:, c:c + 1], in1=hO[c],
                op0=ALU.add, op1=ALU.add)
            a0.append(a0_c)
        a0b = []
        for c in range(DT):
            a0b_c = pool_res.tile([128, 512], BF, name="a0b", tag="resb")
            nc.vector.tensor_copy(a0b_c, a0[c])
            a0b.append(a0b_c)

        aT, aTb = layernorm(a0, a0b, gb1, pool_aT, "aT")

        # -------- FFN --------
        fps = [ps_f.tile([128, 512], F32, name="fps", tag="ps_f")
               for _ in range(DT)]
        for half in range(2):
            u_tiles = []
            for qh in range(2):
                q4 = half * 2 + qh
                w1q = []
                for kt in range(DT):
                    w_t = wtile("w1q")
                    nc.sync.dma_start(
                        w_t, w1_d[l, kt * 128:(kt + 1) * 128,
                                  q4 * 512:(q4 + 1) * 512])
                    w1q.append(w_t)
                for jq in range(4):
                    j = q4 * 4 + jq
                    psu = ps_u.tile([128, 512], F32, name="psu", tag="ps")
                    for kt in range(DT):
                        nc.tensor.matmul(
                            psu, w1q[kt][:, jq * 128:(jq + 1) * 128],
                            aTb[kt], start=(kt == 0), stop=(kt == DT - 1))
                    u_j = pool_u.tile([128, 512], BF, name="u", tag="u")
                    nc.scalar.activation(u_j, psu, AF.Relu, bias=b1_sb[:, j:j + 1])
                    u_tiles.append(u_j)
            for ktw in range(8):
                w2_t = pool_w2.tile([128, D], BF, name="w2t", tag="w2")
                nc.sync.dma_start(
                    w2_t, w2_d[l, (half * 8 + ktw) * 128:
                               (half * 8 + ktw + 1) * 128, :])
                for c in range(DT):
                    nc.tensor.matmul(
                        fps[c], w2_t[:, c * 128:(c + 1) * 128], u_tiles[ktw],
                        start=(half == 0 and ktw == 0),
                        stop=(half == 1 and ktw == 7))
        f0 = []
        for c in range(DT):
            f0_c = pool_res.tile([128, 512], F32, name="f0", tag="res")
            nc.vector.scalar_tensor_tensor(
                out=f0_c, in0=fps[c], scalar=b2_sb[:, c:c + 1], in1=aT[c],
                op0=ALU.add, op1=ALU.add)
            f0.append(f0_c)
        f0b = []
        for c in range(DT):
            f0b_c = pool_res.tile([128, 512], BF, name="f0b", tag="resb")
            nc.vector.tensor_copy(f0b_c, f0[c])
            f0b.append(f0b_c)

        hO, hOb = layernorm(f0, f0b, gb2, pool_hO, "hO")

        # -------- sequence exchange (AllGather within pairs) --------
        if l < L - 1:
            agin = dram.tile([NOWN, D], BF, name=f"agin{l}", tag=f"agin{l}")
            agout = dram.tile([NTOK, D], BF, name=f"agout{l}", tag=f"agout{l}")
            for c in range(DT):
                nc.sync.dma_start(agin[c * 128:(c + 1) * 128, :], hOb[c])
            nc.gpsimd.collective_compute(
                "AllGather", ALU.bypass, replica_groups=PAIRS,
                ins=[agin.opt()], outs=[agout.opt()])
            hF = []
            for c in range(DT):
                hf_c = pool_hF.tile([128, NTOK], BF, name="hF", tag="hF")
                for t in range(2):
                    nc.sync.dma_start(
                        hf_c[:, t * 512:(t + 1) * 512],
                        agout[t * 512 + c * 128: t * 512 + (c + 1) * 128, :])
                hF.append(hf_c)

    # ---------------- output projection ----------------
    ow = []
    for kt in range(DT):
        ow_t = pool_wo.tile([128, CPP], BF, name="ow", tag="wo")
        nc.sync.dma_start(ow_t, ow_d[kt * 128:(kt + 1) * 128, :])
        ow.append(ow_t)
    ob_sb = pool_bias.tile([128, 2], F32, name="ob_sb", tag="ob_sb")
    nc.sync.dma_start(ob_sb[:, 0:1], ob_d[0:128][:, None])
    nc.sync.dma_start(ob_sb[0:64, 1:2], ob_d[128:192][:, None])
    for c, (p0, width) in enumerate(((0, 128), (128, 64))):
        pso = ps_m.tile([128, 512], F32, name="psoo", tag="ps")
        for kt in range(DT):
            nc.tensor.matmul(pso[0:width, :], ow[kt][:, p0:p0 + width],
                             hOb[kt], start=(kt == 0), stop=(kt == DT - 1))
        o_sb = pool_out.tile([128, 512], F32, name="o_sb", tag="outp")
        nc.scalar.activation(o_sb[0:width, :], pso[0:width, :], AF.Identity,
                             bias=ob_sb[0:width, c:c + 1])
        nc.sync.dma_start(out_d[p0:p0 + width, :], o_sb[0:width, :])

    ctx.close()
    tc.__exit__(None, None, None)
    nc.compile()
    return nc


# ---------------- host side ----------------

def _patchify(x, p):
    b, c, hh, ww = x.shape
    x = x.reshape(b, c, hh // p, p, ww // p, p).transpose(0, 2, 4, 1, 3, 5)
    return x.reshape(b, (hh // p) * (ww // p), c * p * p)


def _pos_encoding(n, d):
    pos = np.arange(n, dtype=np.float32)[:, None]
    div = np.exp(np.arange(0, d, 2, dtype=np.float32) * (-np.log(10000.0) / d))
    pe = np.zeros((n, d), np.float32)
    pe[:, 0::2] = np.sin(pos * div)
    pe[:, 1::2] = np.cos(pos * div)
    return pe


_CACHE = {}


def _get_nc():
    if "nc" not in _CACHE:
        nc = bacc.Bacc("TRN2", target_bir_lowering=False, debug=False,
                       num_devices=8)
        _CACHE["nc"] = _build(nc)
    return _CACHE["nc"]


BF16_KEYS = {"patch_wT", "mask_wT", "WqT", "WkT", "WvT", "WoT", "W1T", "W2T",
             "out_wT", "bv", "g1", "be1", "g2", "be2",
             "xpT_f", "xpT_o", "mpT_f", "mpT_o"}


def prep_in_maps(inputs):
    inp = {k: np.ascontiguousarray(np.asarray(v, dtype=np.float32))
           for k, v in inputs.items()}
    xpT = _patchify(inp["x"], PATCH).transpose(0, 2, 1)
    mpT = _patchify(inp["mask"], PATCH).transpose(0, 2, 1)
    pm = _patchify(inp["mask"], PATCH).mean(-1) > 0.5
    pmb = np.where(pm, 0.0, MASK_NEG).astype(np.float32)
    posT = _pos_encoding(NTOK, D).T

    common = {
        "patch_wT": inp["patch_w"].reshape(D, -1).T,
        "mask_wT": inp["mask_w"].reshape(D, -1).T,
        "bias_embed": inp["patch_b"] + inp["mask_b"],
        "WqT": inp["Wq"].transpose(0, 2, 1),
        "WkT": inp["Wk"].transpose(0, 2, 1),
        "WvT": inp["Wv"].transpose(0, 2, 1),
        "WoT": inp["Wo"].transpose(0, 2, 1),
        "W1T": inp["W1"].transpose(0, 2, 1),
        "W2T": inp["W2"].transpose(0, 2, 1),
        "out_wT": inp["out_w"].T,
        "out_b": inp["out_b"],
        "posT_f": posT,
    }
    for k in ("bq", "bk", "bv", "bo", "b1", "b2", "g1", "be1", "g2", "be2"):
        common[k] = inp[k]

    def cvt(k, v):
        dt = ml_dtypes.bfloat16 if k in BF16_KEYS else np.float32
        return np.ascontiguousarray(np.asarray(v).astype(dt))

    common = {k: cvt(k, v) for k, v in common.items()}

    in_maps = []
    for core in range(8):
        b, half = core // 2, core % 2
        tsl = slice(half * NOWN, (half + 1) * NOWN)
        m = dict(common)
        m["xpT_f"] = cvt("xpT_f", xpT[b])
        m["xpT_o"] = cvt("xpT_o", xpT[b][:, tsl])
        m["mpT_f"] = cvt("mpT_f", mpT[b])
        m["mpT_o"] = cvt("mpT_o", mpT[b][:, tsl])
        m["posT_o"] = cvt("posT_o", posT[:, tsl])
        m["pmb"] = pmb[b]
        in_maps.append(m)
    return in_maps


def assemble_output(results):
    o_full = np.zeros((B, CPP, NTOK), np.float32)
    for core in range(8):
        b, half = core // 2, core % 2
        o_full[b][:, half * NOWN:(half + 1) * NOWN] = results[core]["oT"]
    o = o_full.transpose(0, 2, 1)  # [B, N, 192]
    hp = 256 // PATCH
    o = o.reshape(B, hp, hp, 3, PATCH, PATCH).transpose(0, 3, 1, 4, 2, 5)
    return np.ascontiguousarray(o.reshape(B, 3, 256, 256))


def kernel(**inputs):
    nc = _get_nc()
    in_maps = prep_in_maps(inputs)
    res = run_bass_kernel_spmd(nc, in_maps, list(range(8)))
    return assemble_output(res.results)


if __name__ == "__main__":
    _get_nc()
    print("build ok")
